# revision 28
# baseline (speedup 1.0000x reference)
"""Trainium2 Bass kernel for the tiny NeRF MLP (nn_NeRFtinymodel).

Network (per point):
    h1 = relu(emb @ W_in + b_in)            # 32 -> 64
    h2 = relu(h1 @ W0 + b0)                 # 64 -> 64
    x3 = h2 @ Wd + bd                       # 64 -> 16 (no relu)
    dense = x3[:, 0]
    h3 = relu([x3[:,1:], enc_dir] @ Wc + bc)  # (15+39) -> 64
    h4 = relu(h3 @ W1a + b1a)
    h5 = relu(h4 @ W1b + b1b)
    color = h5 @ Wo + bo
    out = [color, dense]

Wd/Wc are algebraically fused on the host (no relu between them):
    h3 = relu(h2 @ (Wd[:,1:]@Wc[:15]) + enc_dir @ Wc[15:] + bc')
    dense = h2 @ Wd[:,0] + bd[0]

Device structure ("variant C"): activations are kept transposed
[features, points], 512 points per tile.  The six matmul stages are
fused pairwise into three K=128/M=128 block-diagonal matmuls per tile,
each combining a stage of tile t with a stage of tile t-2, so every
matmul writes a full PSUM bank at partition base 0 — which is the only
base the float32r ISA path accepts.  float32r streams the PE at 1
column/cycle (bf16 speed) with near-fp32 precision.

    P1(t): rhs=[h3(t-2) | X(t)]      -> [h4pre(t-2) | h1pre(t)]
    P2(t): rhs=[h4(t-2) | h1(t)]     -> [h2pre(t)   | h5pre(t-2)]
    P3(t): rhs=[h2(t)   | h5(t-2)]   -> [h3pre(t) | dense±(t) | color±(t-2)]
           (+ a bf16 rider matmul accumulating enc_dir @ Wc2 onto h3pre)

color/dense use the relu(x) - relu(-x) identity so the shared bias+relu
eltwise op applies to them too; the host reconstructs p - n (exact).
Each bias+relu (PSUM->SBUF) op covers all 128 partitions.  The X input
for tile t is DMA'd into rows 96:128 of the eltwise output tile of
P3(t-2), whose rows 72:128 are zero padding.

Sharding: pure data parallel over 8 cores on the points axis; host
pre-transposes inputs and post-assembles the [N,4] output.
"""

import numpy as np

import concourse.bacc as bacc
import concourse.mybir as mybir
from concourse.tile import TileContext
from concourse.bass_utils import run_bass_kernel_spmd

N_CORES = 8
N_TOTAL = 1048576
NPC = N_TOTAL // N_CORES  # 131072 points per core
F = 512                   # points per tile (one PSUM bank)

f32 = mybir.dt.float32
f32r = mybir.dt.float32r
bf16 = mybir.dt.bfloat16
RELU = mybir.ActivationFunctionType.Relu
ADD = mybir.AluOpType.add
MAX = mybir.AluOpType.max


def build_program(npc=NPC, reps=1):
    assert npc % (2 * F) == 0
    n_tiles = npc // F

    nc = bacc.Bacc("TRN2", target_bir_lowering=False, debug=False,
                   num_devices=N_CORES)
    xT = nc.dram_tensor("xT", [32, npc], f32r, kind="ExternalInput")
    dT = nc.dram_tensor("dT", [64, npc], bf16, kind="ExternalInput")
    wb = nc.dram_tensor("wb", [128, 384], f32r, kind="ExternalInput")
    we = nc.dram_tensor("we", [128, 256], bf16, kind="ExternalInput")
    bb = nc.dram_tensor("bb", [128, 3], f32, kind="ExternalInput")
    oT = nc.dram_tensor("oT", [8, npc], f32r, kind="ExternalOutput")

    with TileContext(nc) as tc:
        with (
            tc.tile_pool(name="wpool", bufs=1) as wpool,
            tc.tile_pool(name="io", bufs=3) as io,
            tc.tile_pool(name="act", bufs=4) as act,
            tc.tile_pool(name="ps", bufs=1, space="PSUM") as ps,
        ):
            wsb = wpool.tile([128, 384], f32r, name="wsb")
            esb = wpool.tile([128, 256], bf16, name="esb")
            bsb = wpool.tile([128, 3], f32, name="bsb")
            nc.sync.dma_start(out=wsb[:], in_=wb[:, :])
            nc.sync.dma_start(out=esb[:], in_=we[:, :])
            nc.sync.dma_start(out=bsb[:], in_=bb[:, :])
            L1 = wsb[:, 0:128]
            L2 = wsb[:, 128:256]
            L3 = wsb[:, 256:384]

            def bias_relu(on_act, dst, src, bias_col):
                b_ap = bsb[:, bias_col:bias_col + 1]
                if on_act:
                    nc.scalar.activation(dst, src, RELU, bias=b_ap, scale=1.0)
                else:
                    nc.vector.tensor_scalar(
                        out=dst, in0=src, scalar1=b_ap, scalar2=0.0,
                        op0=ADD, op1=MAX)

            t3_hist = []   # eltwise-output tiles of P3 (t3_hist[t])
            for rep in range(reps):
                # two zeroed stand-ins for T3(-2), T3(-1)
                t3a = act.tile([128, F], f32r, name="t3a", tag="t3")
                t3b = act.tile([128, F], f32r, name="t3b", tag="t3")
                nc.vector.memset(t3a[:], 0.0)
                nc.vector.memset(t3b[:], 0.0)
                if n_tiles > 0:
                    nc.sync.dma_start(out=t3a[96:128, :], in_=xT[:, 0:F])
                if n_tiles > 1:
                    nc.sync.dma_start(out=t3b[96:128, :], in_=xT[:, F:2 * F])
                t3_hist = [t3a, t3b]
                ds_cur = None
                for t in range(n_tiles + 2):
                    c = t * F
                    live = t < n_tiles          # this tile has real points
                    t3m2 = t3_hist[t]           # T3(t-2)
                    if live and t % 2 == 0:
                        ds_cur = io.tile([128, F], bf16, name="ds", tag="ds")
                        nc.sync.dma_start(out=ds_cur[0:64, :],
                                          in_=dT[:, c:c + F])
                        if t + 1 < n_tiles:
                            nc.sync.dma_start(out=ds_cur[64:128, :],
                                              in_=dT[:, c + F:c + 2 * F])
                        else:
                            nc.sync.dma_start(out=ds_cur[64:128, :],
                                              in_=dT[:, c:c + F])
                    ds = ds_cur

                    b1 = ps.tile([128, F], f32, name="b1", tag="b1", bufs=3)
                    t1 = act.tile([128, F], f32r, name="t1", tag="t1")
                    nc.tensor.matmul(b1[:, :], L1, t3m2[:, :],
                                     start=True, stop=True)
                    bias_relu(t % 2 == 0, t1[:], b1[:], 0)

                    b2 = ps.tile([128, F], f32, name="b2", tag="b2", bufs=3)
                    t2 = act.tile([128, F], f32r, name="t2", tag="t2")
                    nc.tensor.matmul(b2[:, :], L2, t1[:, :],
                                     start=True, stop=True)
                    bias_relu(t % 2 == 1, t2[:], b2[:], 1)

                    b3 = ps.tile([128, F], f32, name="b3", tag="b3", bufs=2)
                    t3 = act.tile([128, F], f32r, name="t3", tag="t3")
                    nc.tensor.matmul(b3[:, :], L3, t2[:, :],
                                     start=True, stop=False)
                    er = esb[:, 0:128] if t % 2 == 0 else esb[:, 128:256]
                    nc.tensor.matmul(b3[:, :], er, ds[:, :],
                                     start=False, stop=True)
                    bias_relu(t % 2 == 0, t3[:], b3[:], 2)
                    t3_hist.append(t3)

                    if live:
                        # dense± of tile t
                        nc.sync.dma_start(out=oT[0:2, c:c + F],
                                          in_=t3[64:66, :])
                    if 0 <= t - 2 < n_tiles:
                        # color± of tile t-2
                        c2 = (t - 2) * F
                        nc.sync.dma_start(out=oT[2:8, c2:c2 + F],
                                          in_=t3[66:72, :])
                    if t + 2 < n_tiles:
                        # X for tile t+2 rides in rows 96:128 of T3(t)
                        c2 = (t + 2) * F
                        nc.sync.dma_start(out=t3[96:128, :],
                                          in_=xT[:, c2:c2 + F])
    nc.compile()
    return nc


def _host_prep(inputs):
    W_in, b_in = inputs["W_in"], inputs["b_in"]
    W0, b0 = inputs["W0"], inputs["b0"]
    Wd, bd = inputs["Wd"], inputs["bd"]
    Wc, bc = inputs["Wc"], inputs["bc"]
    W1a, b1a = inputs["W1a"], inputs["b1a"]
    W1b, b1b = inputs["W1b"], inputs["b1b"]
    Wo, bo = inputs["Wo"], inputs["bo"]

    Wc1 = (Wd[:, 1:].astype(np.float64) @ Wc[:15].astype(np.float64))
    bcp = (bd[1:].astype(np.float64) @ Wc[:15].astype(np.float64)
           + bc.astype(np.float64)).astype(np.float32)

    # fused lhsT blobs ([K, M]; lhsT[k, m] = weight input-k -> output-m)
    wblob = np.zeros((128, 384), np.float32)
    # L1: rows 0:64 = W4 -> cols 0:64 ; rows 96:128 = W_in -> cols 64:128
    wblob[0:64, 0:64] = W1a
    wblob[96:128, 64:128] = W_in
    # L2: rows 0:64 = W5 -> cols 64:128 ; rows 64:128 = W0 -> cols 0:64
    wblob[0:64, 128 + 64:128 + 128] = W1b
    wblob[64:128, 128:128 + 64] = W0
    # L3: rows 0:64 (h2): Wc1 -> cols 0:64, +-Wd0 -> cols 64:66
    #     rows 64:128 (h5): +-Wo -> cols 66:72
    wblob[0:64, 256:256 + 64] = Wc1.astype(np.float32)
    wblob[0:64, 256 + 64] = Wd[:, 0]
    wblob[0:64, 256 + 65] = -Wd[:, 0]
    wblob[64:128, 256 + 66:256 + 69] = Wo
    wblob[64:128, 256 + 69:256 + 72] = -Wo

    # enc_dir rider lhsT (bf16): even tiles contract rows 0:39, odd 64:103
    eblob = np.zeros((128, 256), np.float32)
    eblob[0:39, 0:64] = Wc[15:54]
    eblob[64:103, 128:192] = Wc[15:54]

    bblob = np.zeros((128, 3), np.float32)
    bblob[0:64, 0] = b1a
    bblob[64:128, 0] = b_in
    bblob[0:64, 1] = b0
    bblob[64:128, 1] = b1b
    bblob[0:64, 2] = bcp
    bblob[64, 2] = bd[0]
    bblob[65, 2] = -bd[0]
    bblob[66:69, 2] = bo
    bblob[69:72, 2] = -bo

    np_bf = mybir.dt.np(bf16)
    emb = inputs["emb_points"]
    enc = inputs["enc_dir"]
    in_maps = []
    for cc in range(N_CORES):
        sl = slice(cc * NPC, (cc + 1) * NPC)
        dpad = np.zeros((64, NPC), np_bf)
        dpad[0:39] = np.ascontiguousarray(enc[sl].T).astype(np_bf)
        in_maps.append({
            "xT": np.ascontiguousarray(emb[sl].T),
            "dT": dpad,
            "wb": wblob,
            "we": eblob.astype(np_bf),
            "bb": bblob,
        })
    return in_maps


_PROGRAM_CACHE = {}


def _get_program(npc=NPC, reps=1):
    key = (npc, reps)
    if key not in _PROGRAM_CACHE:
        _PROGRAM_CACHE[key] = build_program(npc, reps)
    return _PROGRAM_CACHE[key]


def kernel(**inputs) -> np.ndarray:
    nc = _get_program(NPC, 1)
    in_maps = _host_prep(inputs)
    res = run_bass_kernel_spmd(nc, in_maps, core_ids=list(range(N_CORES)))
    out = np.empty((N_TOTAL, 4), np.float32)
    for cc in range(N_CORES):
        o = res.results[cc]["oT"]          # [8, NPC]
        sl = slice(cc * NPC, (cc + 1) * NPC)
        out[sl, 3] = o[0] - o[1]           # dense
        out[sl, 0:3] = (o[2:5] - o[5:8]).T  # color
    return out


# revision 29
# speedup vs baseline: 2.4145x; 2.4145x over previous
"""Trainium2 Bass kernel for the tiny NeRF MLP (nn_NeRFtinymodel).

Network (per point):
    h1 = relu(emb @ W_in + b_in)            # 32 -> 64
    h2 = relu(h1 @ W0 + b0)                 # 64 -> 64
    x3 = h2 @ Wd + bd                       # 64 -> 16 (no relu)
    dense = x3[:, 0]
    h3 = relu([x3[:,1:], enc_dir] @ Wc + bc)  # (15+39) -> 64
    h4 = relu(h3 @ W1a + b1a)
    h5 = relu(h4 @ W1b + b1b)
    color = h5 @ Wo + bo
    out = [color, dense]

Wd/Wc are algebraically fused on the host (no relu between them):
    h3 = relu(h2 @ (Wd[:,1:]@Wc[:15]) + enc_dir @ Wc[15:] + bc')
    dense = h2 @ Wd[:,0] + bd[0]

Device structure ("variant C"): activations are kept transposed
[features, points], 512 points per tile.  The six matmul stages are
fused pairwise into three K=128/M=128 block-diagonal matmuls per tile,
each combining a stage of tile t with a stage of tile t-2, so every
matmul writes a full PSUM bank at partition base 0 — which is the only
base the float32r ISA path accepts.  float32r streams the PE at 1
column/cycle (bf16 speed) with near-fp32 precision.

    P1(t): rhs=[h3(t-2) | X(t)]      -> [h4pre(t-2) | h1pre(t)]
    P2(t): rhs=[h4(t-2) | h1(t)]     -> [h2pre(t)   | h5pre(t-2)]
    P3(t): rhs=[h2(t)   | h5(t-2)]   -> [h3pre(t) | dense±(t) | color±(t-2)]
           (+ a bf16 rider matmul accumulating enc_dir @ Wc2 onto h3pre)

color/dense use the relu(x) - relu(-x) identity so the shared bias+relu
eltwise op applies to them too; the host reconstructs p - n (exact).
Each bias+relu (PSUM->SBUF) op covers all 128 partitions.  The X input
for tile t is DMA'd into rows 96:128 of the eltwise output tile of
P3(t-2), whose rows 72:128 are zero padding.

Sharding: pure data parallel over 8 cores on the points axis; host
pre-transposes inputs and post-assembles the [N,4] output.
"""

import numpy as np

import concourse.bacc as bacc
import concourse.mybir as mybir
from concourse.tile import TileContext
from concourse.bass_utils import run_bass_kernel_spmd

N_CORES = 8
N_TOTAL = 1048576
NPC = N_TOTAL // N_CORES  # 131072 points per core
F = 512                   # points per tile (one PSUM bank)

f32 = mybir.dt.float32
f32r = mybir.dt.float32r
bf16 = mybir.dt.bfloat16
RELU = mybir.ActivationFunctionType.Relu
ADD = mybir.AluOpType.add
MAX = mybir.AluOpType.max


def build_program(npc=NPC, reps=1):
    assert npc % (2 * F) == 0
    n_tiles = npc // F

    nc = bacc.Bacc("TRN2", target_bir_lowering=False, debug=False,
                   num_devices=N_CORES)
    xT = nc.dram_tensor("xT", [32, npc], f32r, kind="ExternalInput")
    dT = nc.dram_tensor("dT", [64, npc], bf16, kind="ExternalInput")
    wb = nc.dram_tensor("wb", [128, 384], f32r, kind="ExternalInput")
    we = nc.dram_tensor("we", [128, 256], bf16, kind="ExternalInput")
    bb = nc.dram_tensor("bb", [128, 3], f32, kind="ExternalInput")
    oT = nc.dram_tensor("oT", [8, npc], f32r, kind="ExternalOutput")

    with TileContext(nc) as tc:
        with (
            tc.tile_pool(name="wpool", bufs=1) as wpool,
            tc.tile_pool(name="io", bufs=3) as io,
            tc.tile_pool(name="act", bufs=4) as act,
            tc.tile_pool(name="ps", bufs=1, space="PSUM") as ps,
        ):
            wsb = wpool.tile([128, 384], f32r, name="wsb")
            esb = wpool.tile([128, 256], bf16, name="esb")
            bsb = wpool.tile([128, 3], f32, name="bsb")
            nc.sync.dma_start(out=wsb[:], in_=wb[:, :])
            nc.sync.dma_start(out=esb[:], in_=we[:, :])
            nc.sync.dma_start(out=bsb[:], in_=bb[:, :])
            L1 = wsb[:, 0:128]
            L2 = wsb[:, 128:256]
            L3 = wsb[:, 256:384]

            def bias_relu(on_act, dst, src, bias_col):
                b_ap = bsb[:, bias_col:bias_col + 1]
                if on_act:
                    nc.scalar.activation(dst, src, RELU, bias=b_ap, scale=1.0)
                else:
                    nc.vector.tensor_scalar(
                        out=dst, in0=src, scalar1=b_ap, scalar2=0.0,
                        op0=ADD, op1=MAX)

            t3_hist = []   # eltwise-output tiles of P3 (t3_hist[t])
            for rep in range(reps):
                # two zeroed stand-ins for T3(-2), T3(-1)
                t3a = act.tile([128, F], f32r, name="t3a", tag="t3")
                t3b = act.tile([128, F], f32r, name="t3b", tag="t3")
                nc.vector.memset(t3a[:].bitcast(f32), 0.0)
                nc.vector.memset(t3b[:].bitcast(f32), 0.0)
                if n_tiles > 0:
                    nc.sync.dma_start(out=t3a[96:128, :], in_=xT[:, 0:F])
                if n_tiles > 1:
                    nc.sync.dma_start(out=t3b[96:128, :], in_=xT[:, F:2 * F])
                t3_hist = [t3a, t3b]
                ds_cur = None
                for t in range(n_tiles + 2):
                    c = t * F
                    live = t < n_tiles          # this tile has real points
                    t3m2 = t3_hist[t]           # T3(t-2)
                    if live and t % 2 == 0:
                        ds_cur = io.tile([128, F], bf16, name="ds", tag="ds")
                        nc.sync.dma_start(out=ds_cur[0:64, :],
                                          in_=dT[:, c:c + F])
                        if t + 1 < n_tiles:
                            nc.sync.dma_start(out=ds_cur[64:128, :],
                                              in_=dT[:, c + F:c + 2 * F])
                        else:
                            nc.sync.dma_start(out=ds_cur[64:128, :],
                                              in_=dT[:, c:c + F])
                    ds = ds_cur

                    b1 = ps.tile([128, F], f32, name="b1", tag="b1", bufs=3)
                    t1 = act.tile([128, F], f32r, name="t1", tag="t1")
                    nc.tensor.matmul(b1[:, :], L1, t3m2[:, :],
                                     start=True, stop=True)
                    bias_relu(t % 2 == 0, t1[:], b1[:], 0)

                    b2 = ps.tile([128, F], f32, name="b2", tag="b2", bufs=3)
                    t2 = act.tile([128, F], f32r, name="t2", tag="t2")
                    nc.tensor.matmul(b2[:, :], L2, t1[:, :],
                                     start=True, stop=True)
                    bias_relu(t % 2 == 1, t2[:], b2[:], 1)

                    b3 = ps.tile([128, F], f32, name="b3", tag="b3", bufs=2)
                    t3 = act.tile([128, F], f32r, name="t3", tag="t3")
                    nc.tensor.matmul(b3[:, :], L3, t2[:, :],
                                     start=True, stop=False)
                    er = esb[:, 0:128] if t % 2 == 0 else esb[:, 128:256]
                    nc.tensor.matmul(b3[:, :], er, ds[:, :],
                                     start=False, stop=True)
                    bias_relu(t % 2 == 0, t3[:], b3[:], 2)
                    t3_hist.append(t3)

                    if live:
                        # dense± of tile t
                        nc.sync.dma_start(out=oT[0:2, c:c + F],
                                          in_=t3[64:66, :])
                    if 0 <= t - 2 < n_tiles:
                        # color± of tile t-2
                        c2 = (t - 2) * F
                        nc.sync.dma_start(out=oT[2:8, c2:c2 + F],
                                          in_=t3[66:72, :])
                    if t + 2 < n_tiles:
                        # X for tile t+2 rides in rows 96:128 of T3(t)
                        c2 = (t + 2) * F
                        nc.sync.dma_start(out=t3[96:128, :],
                                          in_=xT[:, c2:c2 + F])
    nc.compile()
    return nc


def _host_prep(inputs):
    W_in, b_in = inputs["W_in"], inputs["b_in"]
    W0, b0 = inputs["W0"], inputs["b0"]
    Wd, bd = inputs["Wd"], inputs["bd"]
    Wc, bc = inputs["Wc"], inputs["bc"]
    W1a, b1a = inputs["W1a"], inputs["b1a"]
    W1b, b1b = inputs["W1b"], inputs["b1b"]
    Wo, bo = inputs["Wo"], inputs["bo"]

    Wc1 = (Wd[:, 1:].astype(np.float64) @ Wc[:15].astype(np.float64))
    bcp = (bd[1:].astype(np.float64) @ Wc[:15].astype(np.float64)
           + bc.astype(np.float64)).astype(np.float32)

    # fused lhsT blobs ([K, M]; lhsT[k, m] = weight input-k -> output-m)
    wblob = np.zeros((128, 384), np.float32)
    # L1: rows 0:64 = W4 -> cols 0:64 ; rows 96:128 = W_in -> cols 64:128
    wblob[0:64, 0:64] = W1a
    wblob[96:128, 64:128] = W_in
    # L2: rows 0:64 = W5 -> cols 64:128 ; rows 64:128 = W0 -> cols 0:64
    wblob[0:64, 128 + 64:128 + 128] = W1b
    wblob[64:128, 128:128 + 64] = W0
    # L3: rows 0:64 (h2): Wc1 -> cols 0:64, +-Wd0 -> cols 64:66
    #     rows 64:128 (h5): +-Wo -> cols 66:72
    wblob[0:64, 256:256 + 64] = Wc1.astype(np.float32)
    wblob[0:64, 256 + 64] = Wd[:, 0]
    wblob[0:64, 256 + 65] = -Wd[:, 0]
    wblob[64:128, 256 + 66:256 + 69] = Wo
    wblob[64:128, 256 + 69:256 + 72] = -Wo

    # enc_dir rider lhsT (bf16): even tiles contract rows 0:39, odd 64:103
    eblob = np.zeros((128, 256), np.float32)
    eblob[0:39, 0:64] = Wc[15:54]
    eblob[64:103, 128:192] = Wc[15:54]

    bblob = np.zeros((128, 3), np.float32)
    bblob[0:64, 0] = b1a
    bblob[64:128, 0] = b_in
    bblob[0:64, 1] = b0
    bblob[64:128, 1] = b1b
    bblob[0:64, 2] = bcp
    bblob[64, 2] = bd[0]
    bblob[65, 2] = -bd[0]
    bblob[66:69, 2] = bo
    bblob[69:72, 2] = -bo

    np_bf = mybir.dt.np(bf16)
    emb = inputs["emb_points"]
    enc = inputs["enc_dir"]
    in_maps = []
    for cc in range(N_CORES):
        sl = slice(cc * NPC, (cc + 1) * NPC)
        dpad = np.zeros((64, NPC), np_bf)
        dpad[0:39] = np.ascontiguousarray(enc[sl].T).astype(np_bf)
        in_maps.append({
            "xT": np.ascontiguousarray(emb[sl].T),
            "dT": dpad,
            "wb": wblob,
            "we": eblob.astype(np_bf),
            "bb": bblob,
        })
    return in_maps


_PROGRAM_CACHE = {}


def _get_program(npc=NPC, reps=1):
    key = (npc, reps)
    if key not in _PROGRAM_CACHE:
        _PROGRAM_CACHE[key] = build_program(npc, reps)
    return _PROGRAM_CACHE[key]


def kernel(**inputs) -> np.ndarray:
    nc = _get_program(NPC, 1)
    in_maps = _host_prep(inputs)
    res = run_bass_kernel_spmd(nc, in_maps, core_ids=list(range(N_CORES)))
    out = np.empty((N_TOTAL, 4), np.float32)
    for cc in range(N_CORES):
        o = res.results[cc]["oT"]          # [8, NPC]
        sl = slice(cc * NPC, (cc + 1) * NPC)
        out[sl, 3] = o[0] - o[1]           # dense
        out[sl, 0:3] = (o[2:5] - o[5:8]).T  # color
    return out


# revision 34
# speedup vs baseline: 3.6019x; 1.4918x over previous
"""Trainium2 Bass kernel for the tiny NeRF MLP (nn_NeRFtinymodel).

Network (per point):
    h1 = relu(emb @ W_in + b_in)            # 32 -> 64
    h2 = relu(h1 @ W0 + b0)                 # 64 -> 64
    x3 = h2 @ Wd + bd                       # 64 -> 16 (no relu)
    dense = x3[:, 0]
    h3 = relu([x3[:,1:], enc_dir] @ Wc + bc)  # (15+39) -> 64
    h4 = relu(h3 @ W1a + b1a)
    h5 = relu(h4 @ W1b + b1b)
    color = h5 @ Wo + bo
    out = [color, dense]

Wd/Wc are algebraically fused on the host (no relu between them):
    h3 = relu(h2 @ (Wd[:,1:]@Wc[:15]) + enc_dir @ Wc[15:] + bc')
    dense = h2 @ Wd[:,0] + bd[0]

Device structure ("variant C2"): activations kept transposed
[features, points], 512 points per tile, tiles processed in groups of
4 with a 4-tile software-pipeline skew.  The six matmul stages fuse
pairwise into three K=128/M=128 block-diagonal float32r matmuls per
tile (float32r = 1 column/cycle at near-fp32 precision; its ISA only
allows PSUM output base 0, which this layout satisfies):

    P1(t): rhs=[h3(t-4) | outs | X(t)] -> [h4pre(t-4) | h1pre(t)]
    P2(t): rhs=[h4(t-4) | h1(t)]       -> [h2pre(t)   | h5pre(t-4)]
    P3(t): rhs=[h2(t)   | h5(t-4)]     -> [h3pre(t) | den±(t) | col±(t-4)]
           + bf16 rider accumulating enc_dir @ Wc2 onto h3pre

color/dense use relu(x)-relu(-x) so the shared bias+relu eltwise
(always full 128 partitions) covers them; the host subtracts (exact).
All four tiles of a group share one 4-bank PSUM tile for P3 and one
wide eltwise; X / enc_dir / outputs move in 3 large DMAs per group
(HWDGE costs ~625ns per DMA instruction, so instruction count rules).

Sharding: pure data parallel over 8 cores on the points axis.
"""

import numpy as np

import concourse.bacc as bacc
import concourse.mybir as mybir
from concourse.tile import TileContext
from concourse.bass_utils import run_bass_kernel_spmd

N_CORES = 8
N_TOTAL = 1048576
NPC = N_TOTAL // N_CORES  # 131072 points per core
F = 512                   # points per tile (one PSUM bank)
G = 4                     # tiles per group (= pipeline skew)

f32 = mybir.dt.float32
f32r = mybir.dt.float32r
bf16 = mybir.dt.bfloat16
RELU = mybir.ActivationFunctionType.Relu
ADD = mybir.AluOpType.add
MAX = mybir.AluOpType.max


def build_program(npc=NPC, reps=1):
    assert npc % (G * F) == 0
    n_groups = npc // (G * F)

    nc = bacc.Bacc("TRN2", target_bir_lowering=False, debug=False,
                   num_devices=N_CORES)
    xT = nc.dram_tensor("xT", [32, npc], f32r, kind="ExternalInput")
    dT = nc.dram_tensor("dT", [128, npc // 2], bf16, kind="ExternalInput")
    wb = nc.dram_tensor("wb", [128, 384], f32r, kind="ExternalInput")
    we = nc.dram_tensor("we", [128, 256], bf16, kind="ExternalInput")
    bb = nc.dram_tensor("bb", [128, 3], f32, kind="ExternalInput")
    oT = nc.dram_tensor("oT", [8, npc], f32r, kind="ExternalOutput")

    with TileContext(nc) as tc:
        with (
            tc.tile_pool(name="wpool", bufs=1) as wpool,
            tc.tile_pool(name="io", bufs=2) as io,
            tc.tile_pool(name="act", bufs=3) as act,
            tc.tile_pool(name="ps", bufs=1, space="PSUM") as ps,
        ):
            wsb = wpool.tile([128, 384], f32r, name="wsb")
            esb = wpool.tile([128, 256], bf16, name="esb")
            bsb = wpool.tile([128, 3], f32, name="bsb")
            nc.sync.dma_start(out=wsb[:], in_=wb[:, :])
            nc.sync.dma_start(out=esb[:], in_=we[:, :])
            nc.sync.dma_start(out=bsb[:], in_=bb[:, :])
            L1 = wsb[:, 0:128]
            L2 = wsb[:, 128:256]
            L3 = wsb[:, 256:384]

            def bias_relu(on_act, dst, src, bias_col):
                b_ap = bsb[:, bias_col:bias_col + 1]
                if on_act:
                    nc.scalar.activation(dst, src, RELU, bias=b_ap, scale=1.0)
                else:
                    nc.vector.tensor_scalar(
                        out=dst, in0=src, scalar1=b_ap, scalar2=0.0,
                        op0=ADD, op1=MAX)

            for rep in range(reps):
                # two zeroed stand-ins for t3w(-2), t3w(-1); pipeline skew is
                # 2 groups (8 tiles) so E3 of group g-1 overlaps group g.
                t3q = []
                for i in range(2):
                    t3p = act.tile([128, G * F], f32r, name="t3p", tag="t3", bufs=4)
                    nc.vector.memset(t3p[:].bitcast(f32), 0.0)
                    if i * G * F < npc:
                        nc.sync.dma_start(
                            out=t3p[96:128, :],
                            in_=xT[:, i * G * F:(i + 1) * G * F])
                    t3q.append(t3p)
                for g in range(n_groups + 2):
                    live = g < n_groups
                    t3p = t3q[g]
                    if live:
                        ds = io.tile([128, 2 * F], bf16, name="ds", tag="ds")
                        nc.sync.dma_start(
                            out=ds[:],
                            in_=dT[:, 2 * g * F:(2 * g + 2) * F])
                    b3w = ps.tile([128, G * F], f32, name="b3w", tag="b3")
                    t3w = act.tile([128, G * F], f32r, name="t3w", tag="t3", bufs=4)
                    for k in range(G):
                        rhs1 = t3p[:, k * F:(k + 1) * F]
                        b1 = ps.tile([128, F], f32, name="b1", tag="b1",
                                     bufs=2)
                        t1 = act.tile([128, F], f32r, name="t1", tag="t1")
                        nc.tensor.matmul(b1[:, :], L1, rhs1,
                                         start=True, stop=True)
                        bias_relu(k in (0, 2), t1[:], b1[:], 0)
                        b2 = ps.tile([128, F], f32, name="b2", tag="b2",
                                     bufs=2)
                        t2 = act.tile([128, F], f32r, name="t2", tag="t2")
                        nc.tensor.matmul(b2[:, :], L2, t1[:, :],
                                         start=True, stop=True)
                        bias_relu(k == 0, t2[:], b2[:], 1)
                        b3s = b3w[:, k * F:(k + 1) * F]
                        if live:
                            nc.tensor.matmul(b3s, L3, t2[:, :],
                                             start=True, stop=False)
                            er = esb[:, 128 * (k % 2):128 * (k % 2) + 128]
                            dslot = ds[:, (k // 2) * F:(k // 2 + 1) * F]
                            nc.tensor.matmul(b3s, er, dslot,
                                             start=False, stop=True)
                        else:
                            nc.tensor.matmul(b3s, L3, t2[:, :],
                                             start=True, stop=True)
                    bias_relu(True, t3w[:], b3w[:], 2)   # wide op on ACT
                    if live:
                        nc.sync.dma_start(
                            out=oT[:, g * G * F:(g + 1) * G * F],
                            in_=t3w[64:72, :])
                    else:
                        c0 = (g - n_groups) * G * F
                        nc.sync.dma_start(out=oT[2:8, c0:c0 + G * F],
                                          in_=t3w[66:72, :])
                    if g + 2 < n_groups:
                        nc.sync.dma_start(
                            out=t3w[96:128, :],
                            in_=xT[:, (g + 2) * G * F:(g + 3) * G * F])
                    t3q.append(t3w)
    nc.compile()
    return nc


def _host_prep(inputs):
    W_in, b_in = inputs["W_in"], inputs["b_in"]
    W0, b0 = inputs["W0"], inputs["b0"]
    Wd, bd = inputs["Wd"], inputs["bd"]
    Wc, bc = inputs["Wc"], inputs["bc"]
    W1a, b1a = inputs["W1a"], inputs["b1a"]
    W1b, b1b = inputs["W1b"], inputs["b1b"]
    Wo, bo = inputs["Wo"], inputs["bo"]

    Wc1 = (Wd[:, 1:].astype(np.float64) @ Wc[:15].astype(np.float64))
    bcp = (bd[1:].astype(np.float64) @ Wc[:15].astype(np.float64)
           + bc.astype(np.float64)).astype(np.float32)

    wblob = np.zeros((128, 384), np.float32)
    # L1: rows 0:64 = W1a (h3->h4pre) -> cols 0:64 ;
    #     rows 96:128 = W_in (X->h1pre) -> cols 64:128
    wblob[0:64, 0:64] = W1a
    wblob[96:128, 64:128] = W_in
    # L2: rows 0:64 = W1b (h4->h5pre) -> cols 64:128 ;
    #     rows 64:128 = W0 (h1->h2pre) -> cols 0:64
    wblob[0:64, 128 + 64:128 + 128] = W1b
    wblob[64:128, 128:128 + 64] = W0
    # L3: rows 0:64 (h2): Wc1 -> cols 0:64, +-Wd0 -> cols 64:66
    #     rows 64:128 (h5): +-Wo -> cols 66:72
    wblob[0:64, 256:256 + 64] = Wc1.astype(np.float32)
    wblob[0:64, 256 + 64] = Wd[:, 0]
    wblob[0:64, 256 + 65] = -Wd[:, 0]
    wblob[64:128, 256 + 66:256 + 69] = Wo
    wblob[64:128, 256 + 69:256 + 72] = -Wo

    # enc rider lhsT (bf16): even tiles contract rows 0:39, odd 39:78
    eblob = np.zeros((128, 256), np.float32)
    eblob[0:39, 0:64] = Wc[15:54]
    eblob[39:78, 128:192] = Wc[15:54]

    bblob = np.zeros((128, 3), np.float32)
    bblob[0:64, 0] = b1a
    bblob[64:128, 0] = b_in
    bblob[0:64, 1] = b0
    bblob[64:128, 1] = b1b
    bblob[0:64, 2] = bcp
    bblob[64, 2] = bd[0]
    bblob[65, 2] = -bd[0]
    bblob[66:69, 2] = bo
    bblob[69:72, 2] = -bo

    np_bf = mybir.dt.np(bf16)
    emb = inputs["emb_points"]
    enc = inputs["enc_dir"]
    in_maps = []
    for cc in range(N_CORES):
        sl = slice(cc * NPC, (cc + 1) * NPC)
        encT = np.ascontiguousarray(enc[sl].T).astype(np_bf)  # [39, NPC]
        # pair-interleaved, zero-padded enc blob [128, NPC//2]
        dpad = np.zeros((128, NPC // 2), np_bf)
        e4 = encT.reshape(39, NPC // (2 * F), 2, F)
        dpad[0:39] = e4[:, :, 0, :].reshape(39, NPC // 2)
        dpad[39:78] = e4[:, :, 1, :].reshape(39, NPC // 2)
        in_maps.append({
            "xT": np.ascontiguousarray(emb[sl].T),
            "dT": dpad,
            "wb": wblob,
            "we": eblob.astype(np_bf),
            "bb": bblob,
        })
    return in_maps


_PROGRAM_CACHE = {}


def _get_program(npc=NPC, reps=1):
    key = (npc, reps)
    if key not in _PROGRAM_CACHE:
        _PROGRAM_CACHE[key] = build_program(npc, reps)
    return _PROGRAM_CACHE[key]


def kernel(**inputs) -> np.ndarray:
    nc = _get_program(NPC, 1)
    in_maps = _host_prep(inputs)
    res = run_bass_kernel_spmd(nc, in_maps, core_ids=list(range(N_CORES)))
    out = np.empty((N_TOTAL, 4), np.float32)
    for cc in range(N_CORES):
        o = res.results[cc]["oT"]          # [8, NPC]
        sl = slice(cc * NPC, (cc + 1) * NPC)
        out[sl, 3] = o[0] - o[1]           # dense
        # color of tile t is stored at tile slot t+8 (mod n_tiles)
        col = o[2:5] - o[5:8]              # [3, NPC]
        out[sl, 0:3] = np.roll(col, -2 * G * F, axis=1).T
    return out
